# revision 9
# baseline (speedup 1.0000x reference)
"""Sparse multi-head attention (nn_MultiHeadAttention_44332652429419) on 8 trn2 cores.

Strategy (tensor-parallel over H=16 heads, 2 heads per core):
  Host: compose the two stacked linear layers (q/k/v_proj followed by
        MultiheadAttention in_proj) into one weight per tensor; build the
        dense multiplicative mask exp(additive_mask) transposed; pre-tile
        x and the weights into [partition, k, ...] layout so every DMA is
        contiguous per partition. All matmul operands bf16.
  Device (per core, SPMD with per-core weight slices):
    q2T/k2T/v2T = W_c @ x.T + b_c            [128, 3072] (2 heads x 64 dims)
    scoresT[mk,nq] = k2zT_h.T-slice @ q2T    (K=128, dead half zeroed)
    p = exp(scoresT * 1/8) (ACT, bf16 out); pm = p * maskT (DVE 16-bit)
    outT_aug = [v_h | 1].T @ pm              (rowsum via ones-augmented V)
    attnT = outT[:64] * recip(outT[64])      (per-512 half epilogue)
    ypart = attnT.T-slices @ woT_c           (partial out_proj, K=128)
  Host: y = sum_c ypart_c + bo
"""
import os
import sys

sys.path.insert(0, "/opt/trn_rl_repo")

import numpy as np
import ml_dtypes
from contextlib import ExitStack

import concourse.bass as bass
import concourse.bacc as bacc
import concourse.mybir as mybir
import concourse.tile as tile
from concourse.bass_utils import run_bass_kernel_spmd
from concourse.masks import make_identity

F32 = mybir.dt.float32
BF16 = mybir.dt.bfloat16
AF = mybir.ActivationFunctionType
ALU = mybir.AluOpType

N = 3072
IN_F = 1024
OUT_F = 1024
H = 16
D = 64
NCORES = 8
HPC = H // NCORES            # heads per core = 2
CW = HPC * D                 # per-core width = 128
P = 128
NT = N // P                  # 24 key tiles
KT = IN_F // P               # 8 contraction tiles
SQ = 1024                    # query strip width (phase B)
NSQ = N // SQ                # 3 strips
SP = 512                     # proj strip width / epilogue half width
NSP = N // SP                # 6 strips
SCALE = 1.0 / 8.0            # 1/sqrt(D)
RES_MK = 12                  # key tiles whose mask row-block stays resident


def build_program():
    nc = bacc.Bacc()
    # pre-tiled [partition, k, ...] layouts — contiguous per partition row
    xT = nc.declare_dram_parameter("xT", [P, KT * N], BF16, isOutput=False)
    maskT = nc.declare_dram_parameter("maskT", [N, N], BF16, isOutput=False)
    wqT = nc.declare_dram_parameter("wqT", [P, KT * CW], BF16, isOutput=False)
    wkT = nc.declare_dram_parameter("wkT", [P, KT * CW], BF16, isOutput=False)
    wvT = nc.declare_dram_parameter("wvT", [P, KT * CW], BF16, isOutput=False)
    bq = nc.declare_dram_parameter("bq", [CW], F32, isOutput=False)
    bk = nc.declare_dram_parameter("bk", [CW], F32, isOutput=False)
    bv = nc.declare_dram_parameter("bv", [CW], F32, isOutput=False)
    woT = nc.declare_dram_parameter("woT", [CW, OUT_F], BF16, isOutput=False)
    ypart = nc.declare_dram_parameter("ypart", [N, OUT_F], BF16, isOutput=True)

    xTt = xT.rearrange("p (k n) -> p k n", k=KT)

    with tile.TileContext(nc) as tc, ExitStack() as ctx:
        cst = ctx.enter_context(tc.tile_pool(name="cst", bufs=1))
        lp = ctx.enter_context(tc.tile_pool(name="lp", bufs=2))       # xs/v2Ts
        wp = ctx.enter_context(tc.tile_pool(name="wp", bufs=3))       # mask tiles
        pq = ctx.enter_context(tc.tile_pool(name="pq", bufs=5))       # p/pm queue
        ep = ctx.enter_context(tc.tile_pool(name="ep", bufs=2))       # epilogue
        yp = ctx.enter_context(tc.tile_pool(name="yp", bufs=2))       # out staging
        pp = ctx.enter_context(tc.tile_pool(name="pp", bufs=2, space="PSUM"))
        pso = ctx.enter_context(tc.tile_pool(name="pso", bufs=1, space="PSUM"))

        # startup order: first q-projection matmul needs wq + xs0_a[k=0,1]
        wq_sb = cst.tile([P, KT, CW], BF16)
        nc.sync.dma_start(wq_sb[:], wqT.rearrange("p (k m) -> p k m", k=KT))
        xs0_a = lp.tile([P, KT // 2, SP], BF16, tag="xs_a", name="xs_a")
        nc.sync.dma_start(xs0_a[:, 0:2, :], xTt[:, 0:2, 0:SP])
        nc.sync.dma_start(xs0_a[:, 2:4, :], xTt[:, 2:4, 0:SP])
        bq_sb = cst.tile([P, 1], F32)
        nc.sync.dma_start(bq_sb[:], bq[:, None])
        xs0_b = lp.tile([P, KT // 2, SP], BF16, tag="xs_b", name="xs_b")
        nc.sync.dma_start(xs0_b[:], xTt[:, KT // 2:KT, 0:SP])
        wk_sb = cst.tile([P, KT, CW], BF16)
        nc.sync.dma_start(wk_sb[:], wkT.rearrange("p (k m) -> p k m", k=KT))
        bk_sb = cst.tile([P, 1], F32)
        nc.sync.dma_start(bk_sb[:], bk[:, None])
        wv_sb = cst.tile([P, KT, CW], BF16)
        nc.sync.dma_start(wv_sb[:], wvT.rearrange("p (k m) -> p k m", k=KT))
        bv_sb = cst.tile([P, 1], F32)
        nc.sync.dma_start(bv_sb[:], bv[:, None])
        wo_sb = cst.tile([P, OUT_F], BF16)
        nc.sync.dma_start(wo_sb[:], woT[:])

        ident = cst.tile([P, P], F32)
        make_identity(nc, ident)

        # per-512-strip persistent tensors; both heads row-packed.
        # k2z[h][s]: only rows h*D..h*D+63 live, rest zero — score matmuls
        # contract over full K=128 (partition-offset matmul operands
        # miscompute on hw, so keep operands at partition base 0)
        q2s = [cst.tile([P, SP], BF16, tag=f"q2s{s}", name=f"q2s{s}")
               for s in range(NSP)]
        k2zs = [[cst.tile([P, SP], BF16, tag=f"k2z{h}_{s}", name=f"k2z{h}_{s}")
                 for s in range(NSP)] for h in range(HPC)]
        attn_s = [cst.tile([P, SQ], BF16, tag=f"attn{q}", name=f"attn{q}")
                  for q in range(NSQ)]
        vaug = [cst.tile([P, NT, D + 1], BF16, tag=f"vaug{h}", name=f"vaug{h}")
                for h in range(HPC)]
        ones_col = cst.tile([P, 1], F32)
        nc.vector.memset(ones_col[:], 1.0)
        ones_row = cst.tile([1, D], F32)
        nc.vector.memset(ones_row[:], 1.0)
        zero_col = cst.tile([P, 1], F32)
        nc.vector.memset(zero_col[:], 0.0)
        for h in range(HPC):
            nc.vector.tensor_copy(vaug[h][:, :, D:D + 1],
                                  ones_col[:, 0:1, None].to_broadcast([P, NT, 1]))
            osl = slice((1 - h) * D, (2 - h) * D)   # the dead half of k2z[h]
            for s in range(NSP):
                nc.vector.tensor_copy(k2zs[h][s][osl, :],
                                      zero_col[osl, 0:1].to_broadcast([D, SP]))

        # ---- emission helpers ----
        def emit_proj_strip(s):
            if s == 0:
                xs_a, xs_b = xs0_a, xs0_b
            else:
                xs_a = lp.tile([P, KT // 2, SP], BF16, tag="xs_a", name="xs_a")
                nc.sync.dma_start(xs_a[:],
                                  xTt[:, 0:KT // 2, s * SP:(s + 1) * SP])
                xs_b = lp.tile([P, KT // 2, SP], BF16, tag="xs_b", name="xs_b")
                nc.sync.dma_start(xs_b[:],
                                  xTt[:, KT // 2:KT, s * SP:(s + 1) * SP])
            def xsk(k):
                return xs_a[:, k, :] if k < KT // 2 else xs_b[:, k - KT // 2, :]
            ps = pp.tile([P, SQ], F32, tag="ps_s", name="ps_q")
            for k in range(KT):
                nc.tensor.matmul(ps[:, 0:SP], wq_sb[:, k, :], xsk(k),
                                 start=(k == 0), stop=(k == KT - 1))
            nc.vector.tensor_scalar_add(q2s[s][:], ps[:, 0:SP], bq_sb[:, 0:1])
            ps = pp.tile([P, SQ], F32, tag="ps_s", name="ps_k")
            for k in range(KT):
                nc.tensor.matmul(ps[:, 0:SP], wk_sb[:, k, :], xsk(k),
                                 start=(k == 0), stop=(k == KT - 1))
            for h in range(HPC):
                hsl = slice(h * D, (h + 1) * D)
                nc.vector.tensor_scalar_add(k2zs[h][s][hsl, :], ps[hsl, 0:SP],
                                            bk_sb[hsl, 0:1])
            # v: project then transpose into vaug
            ps = pp.tile([P, SQ], F32, tag="ps_s", name="ps_v")
            for k in range(KT):
                nc.tensor.matmul(ps[:, 0:SP], wv_sb[:, k, :], xsk(k),
                                 start=(k == 0), stop=(k == KT - 1))
            v2Ts = lp.tile([P, SP], F32, tag="v2Ts", name="v2Ts")
            nc.vector.tensor_scalar_add(v2Ts[:], ps[:, 0:SP], bv_sb[:, 0:1])
            for b in range(SP // P):
                t = s * (SP // P) + b
                ps_t = pp.tile([P, SQ], F32, tag="ps_s", name="ps_t")
                nc.tensor.transpose(ps_t[:, 0:P], v2Ts[:, b * P:(b + 1) * P],
                                    ident[:])
                for h in range(HPC):
                    nc.vector.tensor_copy(vaug[h][:, t, 0:D],
                                          ps_t[:, h * D:h * D + D])

        def emit_pv(ps_o, h, mk, p, halves=(0, 1)):
            for half in halves:
                fsl = slice(half * SP, (half + 1) * SP)
                nc.tensor.matmul(
                    ps_o[h][half][:, :],
                    vaug[h][:, mk, :],
                    p[:, fsl],
                    start=(mk == 0), stop=(mk == NT - 1),
                )

        res_masks = {}

        def emit_attn_tiles(sq, ps_o, mks, pend):
            for mk in mks:
                if mk < RES_MK:
                    if mk not in res_masks:
                        rm = cst.tile([P, N], BF16, tag=f"rm{mk}", name=f"rm{mk}")
                        nc.sync.dma_start(rm[:], maskT[mk * P:(mk + 1) * P, :])
                        res_masks[mk] = rm
                    mt = res_masks[mk][:, sq * SQ:(sq + 1) * SQ]
                else:
                    mtt = wp.tile([P, SQ], BF16, tag="mt", name="mt")
                    nc.sync.dma_start(
                        mtt[:], maskT[mk * P:(mk + 1) * P, sq * SQ:(sq + 1) * SQ])
                    mt = mtt[:]
                for h in range(HPC):
                    ps_s = pp.tile([P, SQ], F32, tag="ps_s", name="ps_s")
                    for half in range(SQ // SP):
                        fsl = slice(half * SP, (half + 1) * SP)
                        nc.tensor.matmul(
                            ps_s[:, fsl],
                            k2zs[h][mk // 4][:, (mk % 4) * P:(mk % 4 + 1) * P],
                            q2s[sq * (SQ // SP) + half][:],
                            start=True, stop=True,
                        )
                    p = pq.tile([P, SQ], BF16, tag="p", name="p")
                    nc.scalar.activation(p[:], ps_s[:], AF.Exp, scale=SCALE)
                    pm = pq.tile([P, SQ], BF16, tag="pm", name="pm")
                    nc.vector.tensor_tensor(pm[:], p[:], mt, ALU.mult)
                    # software-pipeline: defer this tile's PV until after the
                    # next tile's scores so the PE stream never head-of-line
                    # blocks on the exp
                    pend.append((h, mk, pm))
                    if len(pend) > 3:
                        emit_pv(ps_o, *pend.pop(0))

        def emit_stage(ps_o, half, osbs):
            # drain the finished PSUM half to SBUF ASAP so the next strip's
            # PV can take the bank back
            for h in range(HPC):
                osb = ep.tile([D + 1, SP], F32, tag=f"osb{h}_{half}",
                              name=f"osb{h}_{half}")
                nc.vector.tensor_copy(osb[:], ps_o[h][half][:])
                osbs[(h, half)] = osb

        def emit_norm(sq, half, osbs, act_copies=0):
            # normalize: recip of the ones-row (staged to partition 0 —
            # custom-DVE ops ignore input partition offsets on hw), gpsimd
            # partition-broadcast, one [64, SP] multiply per head, then
            # out_proj + output staging for this half's node tiles
            fsl = slice(half * SP, (half + 1) * SP)
            for h in range(HPC):
                osb = osbs[(h, half)]
                zrow = ep.tile([1, SP], F32, tag="zrow", name=f"zrow{h}")
                nc.vector.tensor_copy(zrow[:], osb[D:D + 1, :])
                recip = ep.tile([1, SP], F32, tag="recip", name=f"recip{h}")
                nc.vector.reciprocal_approx_fast(recip[:], zrow[:])
                # broadcast 1/den across the 64 head dims on the PE (K=1
                # matmul with a ones stationary) — faster than gpsimd
                ps_bc = pp.tile([P, SQ], F32, tag="ps_s", name="ps_bc")
                nc.tensor.matmul(ps_bc[0:D, 0:SP], ones_row[:], recip[:],
                                 start=True, stop=True)
                nc.vector.tensor_tensor(
                    attn_s[sq][h * D:(h + 1) * D, fsl],
                    osb[0:D, :], ps_bc[0:D, 0:SP], ALU.mult)
            for b in range(half * (SP // P), (half + 1) * (SP // P)):
                t = sq * (SQ // P) + b
                ps_y = pp.tile([P, SQ], F32, tag="ps_s", name="ps_y")
                for f in range(OUT_F // SP):
                    nc.tensor.matmul(ps_y[:, f * SP:(f + 1) * SP],
                                     attn_s[sq][:, b * P:(b + 1) * P],
                                     wo_sb[:, f * SP:(f + 1) * SP],
                                     start=True, stop=True)
                ys = yp.tile([P, OUT_F], BF16, tag="ys", name="ys")
                if b % (SP // P) >= (SP // P) - act_copies:
                    nc.scalar.copy(ys[:], ps_y[:])
                else:
                    nc.vector.tensor_copy(ys[:], ps_y[:])
                nc.sync.dma_start(ypart[t * P:(t + 1) * P, :], ys[:])

        def make_pso(sq):
            return [[pso.tile([D + 1, SP], F32, tag=f"ps_o{h}_{half}",
                              name=f"ps_o{h}_{half}_{sq}")
                     for half in range(SQ // SP)] for h in range(HPC)]

        # ---- interleaved emission: A strips feed B(sq=0) chunks ----
        emit_proj_strip(0)
        emit_proj_strip(1)
        ps_o0 = make_pso(0)
        pend0 = []
        emit_attn_tiles(0, ps_o0, range(0, 8), pend0)
        emit_proj_strip(2)
        emit_attn_tiles(0, ps_o0, range(8, 12), pend0)
        emit_proj_strip(3)
        emit_attn_tiles(0, ps_o0, range(12, 16), pend0)
        emit_proj_strip(4)
        emit_attn_tiles(0, ps_o0, range(16, 20), pend0)
        emit_proj_strip(5)
        emit_attn_tiles(0, ps_o0, range(20, 24), pend0)
        ps_o_cur, pend_cur = ps_o0, pend0
        for sq in range(1, NSQ):
            ps_o_nxt = make_pso(sq)
            pend_nxt = []
            osbs = {}
            emit_attn_tiles(sq, ps_o_nxt, range(0, 2), pend_nxt)
            # finish the previous strip's PV half-by-half and stage each
            # half out of PSUM immediately
            for args in pend_cur:
                emit_pv(ps_o_cur, *args, halves=(0,))
            emit_stage(ps_o_cur, 0, osbs)
            for args in pend_cur:
                emit_pv(ps_o_cur, *args, halves=(1,))
            emit_stage(ps_o_cur, 1, osbs)
            emit_attn_tiles(sq, ps_o_nxt, range(2, 8), pend_nxt)
            emit_norm(sq - 1, 0, osbs)
            emit_attn_tiles(sq, ps_o_nxt, range(8, 16), pend_nxt)
            emit_norm(sq - 1, 1, osbs)
            emit_attn_tiles(sq, ps_o_nxt, range(16, NT), pend_nxt)
            ps_o_cur, pend_cur = ps_o_nxt, pend_nxt
        osbs = {}
        for args in pend_cur:
            emit_pv(ps_o_cur, *args, halves=(0,))
        emit_stage(ps_o_cur, 0, osbs)
        for args in pend_cur:
            emit_pv(ps_o_cur, *args, halves=(1,))
        emit_norm(NSQ - 1, 0, osbs, act_copies=4)
        emit_stage(ps_o_cur, 1, osbs)
        emit_norm(NSQ - 1, 1, osbs, act_copies=4)

    nc.compile()
    return nc


_PROGRAM = None
LAST_RESULTS = None


def _get_program():
    global _PROGRAM
    if _PROGRAM is None:
        _PROGRAM = build_program()
    return _PROGRAM


def _softplus(x):
    x = np.asarray(x, np.float32)
    return np.logaddexp(0.0, x).astype(np.float32)


def host_prep(inputs):
    x = np.asarray(inputs["x"], np.float32)
    edge_index = np.asarray(inputs["edge_index"])
    edge_type = np.asarray(inputs["edge_type"])
    etw = np.asarray(inputs["edge_type_weights"], np.float32)

    def f32(k):
        return np.asarray(inputs[k], np.float32)

    # compose the two linear layers: q2 = x @ (wiq@wq).T + (wiq@bq + biq)
    WQ = f32("wiq") @ f32("wq")
    bQ = f32("wiq") @ f32("bq") + f32("biq")
    WK = f32("wik") @ f32("wk")
    bK = f32("wik") @ f32("bk") + f32("bik")
    WV = f32("wiv") @ f32("wv")
    bV = f32("wiv") @ f32("bv") + f32("biv")
    wo = f32("wo")
    bo = f32("bo")

    # multiplicative mask, transposed: maskT[m, n] = exp(add_mask[n, m])
    w = _softplus(etw)
    NEG = np.float32(-60000.0)
    M = np.full((N, N), NEG, dtype=np.float32)
    src, dst = edge_index[0], edge_index[1]
    M[src, dst] = w[edge_type - 1]             # last write wins, like jax .at[].set
    diag = np.diagonal(M).copy()
    didx = np.arange(N)
    M[didx, didx] = np.where(diag == NEG, w[3], diag)

    bf = ml_dtypes.bfloat16
    maskT = np.exp(np.ascontiguousarray(M.T)).astype(bf)

    # pre-tile into [partition, k, ...] so DMA segments are contiguous
    def ptile(a):   # [KT*P, M] -> [P, KT*M]
        kt = a.shape[0] // P
        return np.ascontiguousarray(
            a.reshape(kt, P, a.shape[1]).transpose(1, 0, 2).reshape(P, -1))

    xTt = ptile(np.ascontiguousarray(x.T)).astype(bf)

    in_maps = []
    for c in range(NCORES):
        rs = slice(c * CW, (c + 1) * CW)
        in_maps.append({
            "xT": xTt,
            "maskT": maskT,
            "wqT": ptile(np.ascontiguousarray(WQ[rs].T)).astype(bf),
            "wkT": ptile(np.ascontiguousarray(WK[rs].T)).astype(bf),
            "wvT": ptile(np.ascontiguousarray(WV[rs].T)).astype(bf),
            "bq": np.ascontiguousarray(bQ[rs]),
            "bk": np.ascontiguousarray(bK[rs]),
            "bv": np.ascontiguousarray(bV[rs]),
            "woT": np.ascontiguousarray(wo[:, rs].T).astype(bf),
        })
    return in_maps, bo


def kernel(**inputs) -> np.ndarray:
    global LAST_RESULTS
    in_maps, bo = host_prep(inputs)
    nc = _get_program()
    trace = bool(os.environ.get("KERNEL_TRACE"))
    res = run_bass_kernel_spmd(nc, in_maps, list(range(NCORES)), trace=trace)
    LAST_RESULTS = res
    y = bo[None, :].astype(np.float32).repeat(N, axis=0)
    for c in range(NCORES):
        y += res.results[c]["ypart"].astype(np.float32)
    return y
